# revision 4
# baseline (speedup 1.0000x reference)
"""Causal dilated 1D conv (B=16, C=32, L=131072, KW=3, dil=4, left-pad 8)
as a Bass/Tile kernel on 8 Trainium2 NeuronCores — v4: direct fp8e3 feed.

Key idea: store x host-side as fp8e3 (e3m4) bit patterns (1 byte, ~1.3%
rel quantization error for unit-variance data) and feed the matmuls
DIRECTLY via a bitcast AP — the PE runs fp8e3 moving operands at full
bf16 rate (HW-verified exact).  This eliminates the entire int8->fp16
upcast stage of the baseline (no SWDGE casting DMA, no DVE/ACT/GPSIMD
cast ops, half the SBUF supply traffic).

Per core: 16 half-run tiles x (1 HWDGE in-DMA + 16 matmuls + 4 PSUM
evacs [DVE/ACT alternating] + 1 SWDGE-ring out-DMA).  Weights are a
[128, 256] bf16 stationary pair (W_A | W_B), output is int8 with
per-channel scales folded into the weights (see kernel.py baseline).
"""

import numpy as np
import ml_dtypes

import concourse.bass as bass
import concourse.mybir as mybir
from concourse.tile import TileContext
from concourse.bass_utils import run_bass_kernel_spmd

B, C, L = 16, 32, 131072
KW, DIL, PAD = 3, 4, 8

N_CORES = 8
B_PER_CORE = B // N_CORES          # 2
NPH = DIL                          # 4 phases
JBLK = 4                           # phase positions per block
LPH = L // NPH                     # 32768 phase positions per run
NBLK = LPH // JBLK                 # 8192 block-columns per run
ZCOLS = NBLK + 1                   # input block-columns per run (halo)
RUNS = B_PER_CORE * NPH            # 8 (batch, phase) runs per core
HRUN = NBLK // 2                   # 4096 block-columns per half-run tile
NTILE = RUNS * 2                   # 16 half-run tiles per core
XFREE = RUNS * ZCOLS
OFREE = RUNS * NBLK

S_X = 4.0                          # e3m4 scale: 1 sigma -> 4.0 (max 15.5)
CLIP_Y = 4.0                       # output clip in per-channel sigma units

CHUNK = 1024                       # out cols per chunk (1 evac, 4 MMs)
NCH = HRUN // CHUNK                # 4 chunks per tile

Copy = mybir.ActivationFunctionType.Copy
F8 = mybir.dt.float8e3


def _split_sync_waits(nc: bass.Bass, max_waits: int = 1) -> None:
    """Walrus build rejects >1 sync-waits per instruction; hoist extras
    onto NoOps (program order on the engine preserves semantics)."""
    ctr = 0
    for f in nc.m.functions:
        for bb in f.blocks:
            insts = bb.instructions
            new = []
            for inst in insts:
                si = getattr(inst, "sync_info", None)
                if si is not None and si.on_wait and len(si.on_wait) > max_waits:
                    waits = list(si.on_wait)
                    head, keep = waits[:-max_waits], waits[-max_waits:]
                    for w in head:
                        nop = mybir.InstNoOp(
                            name=f"splitw-{ctr}",
                            engine=inst.engine,
                            bass_nofuse=True,
                            sync_info=mybir.SyncInfo(on_wait=[w], on_update=[]),
                        )
                        ctr += 1
                        new.append(nop)
                    inst.sync_info = mybir.SyncInfo(
                        on_wait=keep, on_update=list(si.on_update or [])
                    )
                new.append(inst)
            insts[:] = new


def _build_nc() -> bass.Bass:
    nc = bass.Bass(target_bir_lowering=False, trn_type="TRN2")
    x = nc.dram_tensor("x", [128, XFREE], mybir.dt.int8, kind="ExternalInput")
    w = nc.dram_tensor("w", [128, 2 * 128], mybir.dt.bfloat16,
                       kind="ExternalInput")
    out = nc.dram_tensor("out", [128, OFREE], mybir.dt.int8,
                         kind="ExternalOutput")

    with TileContext(nc) as tc:
        with (
            tc.tile_pool(name="wpool", bufs=1) as wpool,
            tc.tile_pool(name="x8pool", bufs=5) as x8pool,
            tc.tile_pool(name="opool", bufs=3) as opool,
            tc.tile_pool(name="psum", bufs=4, space="PSUM") as psum,
        ):
            wt = wpool.tile([128, 2 * 128], mybir.dt.bfloat16)
            nc.sync.dma_start(out=wt[:], in_=w[:])

            ev = 0
            for t in range(NTILE):
                run, h = divmod(t, 2)
                xoff = run * ZCOLS + h * HRUN
                ooff = run * NBLK + h * HRUN

                x8 = x8pool.tile([128, HRUN + 1], mybir.dt.int8, name="x8")
                if t == 0:
                    # startup taper: load in 4 pieces so matmuls start early
                    bounds = [0, 513, 1537, 2561, 4097]
                    for p in range(4):
                        lo, hi = bounds[p], bounds[p + 1]
                        nc.sync.dma_start(
                            out=x8[:, lo:hi],
                            in_=bass.AP(x, xoff + lo,
                                        [[XFREE, 128], [1, hi - lo]]),
                        )
                else:
                    nc.sync.dma_start(
                        out=x8[:],
                        in_=bass.AP(x, xoff, [[XFREE, 128], [1, HRUN + 1]]),
                    )
                xf = x8[:].bitcast(F8)

                ot = opool.tile([128, HRUN], mybir.dt.int8, name="ot")
                for c in range(NCH):
                    pt = psum.tile([128, CHUNK], mybir.dt.float32, name="pt")
                    for hh in range(CHUNK // 512):
                        o0 = c * CHUNK + hh * 512
                        nc.tensor.matmul(
                            out=pt[:, hh * 512:hh * 512 + 512],
                            lhsT=wt[:, 0:128],
                            rhs=xf[:, o0:o0 + 512],
                            start=True, stop=False,
                        )
                        nc.tensor.matmul(
                            out=pt[:, hh * 512:hh * 512 + 512],
                            lhsT=wt[:, 128:256],
                            rhs=xf[:, o0 + 1:o0 + 513],
                            start=False, stop=True,
                        )
                    oslice = ot[:, c * CHUNK:(c + 1) * CHUNK]
                    last = (t == NTILE - 1 and c == NCH - 1)
                    if last:
                        nc.vector.tensor_copy(out=oslice[:, 0:512],
                                              in_=pt[:, 0:512])
                        nc.scalar.activation(out=oslice[:, 512:1024],
                                             in_=pt[:, 512:1024], func=Copy)
                    elif ev % 2 == 0:
                        nc.vector.tensor_copy(out=oslice, in_=pt[:])
                    else:
                        nc.scalar.activation(out=oslice, in_=pt[:], func=Copy)
                    ev += 1
                if t == NTILE - 1:
                    for half in range(2):
                        lo = half * 2 * CHUNK
                        nc.sync.dma_start(
                            out=bass.AP(out, ooff + lo,
                                        [[OFREE, 128], [1, 2 * CHUNK]]),
                            in_=ot[:, lo:lo + 2 * CHUNK],
                        )
                else:
                    nc.sync.dma_start(
                        out=bass.AP(out, ooff, [[OFREE, 128], [1, HRUN]]),
                        in_=ot[:],
                    )
    _split_sync_waits(nc)
    return nc


_NC_CACHE = None


def _get_nc() -> bass.Bass:
    global _NC_CACHE
    if _NC_CACHE is None:
        _NC_CACHE = _build_nc()
    return _NC_CACHE


def _pack_weights(W: np.ndarray, s_o: np.ndarray) -> np.ndarray:
    """[128, 256] stationary (W_A | W_B) in bf16, scales folded.

    lhsT[(ci*4+q), (co*4+j)]: W_A holds tap k=q-j, W_B holds tap k=q+4-j."""
    Wf = W.reshape(C, C, KW) * (1.0 / (S_X * s_o))[:, None, None]
    WA = np.zeros((C, JBLK, C, JBLK), np.float32)
    WB = np.zeros((C, JBLK, C, JBLK), np.float32)
    for j in range(JBLK):
        for q in range(JBLK):
            k = q - j
            if 0 <= k < KW:
                WA[:, q, :, j] = Wf[:, :, k].T
            k = q + JBLK - j
            if 0 <= k < KW:
                WB[:, q, :, j] = Wf[:, :, k].T
    return np.concatenate(
        [WA.reshape(128, 128), WB.reshape(128, 128)], axis=1
    ).astype(ml_dtypes.bfloat16)


def kernel(x: np.ndarray, W: np.ndarray, _trace: bool = False):
    x = np.ascontiguousarray(x, dtype=np.float32)   # (16, 32, 131072)
    W = np.ascontiguousarray(W, dtype=np.float32)   # (32, 96)

    # Quantize x to fp8e3 (e3m4) bit patterns; clip to stay finite.
    xs = np.clip(x * S_X, -15.5, 15.5)
    xq = xs.astype(ml_dtypes.float8_e3m4).view(np.int8)

    # Pad so every z position 16b'+4q+r maps to x[pos-8] (0 outside [0,L)).
    xp = np.zeros((B, C, 16 * ZCOLS), dtype=np.int8)
    xp[:, :, PAD:PAD + L] = xq
    v = xp.reshape(B, C, ZCOLS, JBLK, NPH).transpose(0, 1, 3, 4, 2)

    # Per-channel output scale; PSUM = out / (S_X * s_o) * S_X ... folded so
    # PSUM values are int8-ready.
    s_o = CLIP_Y * np.linalg.norm(W, axis=1) / 127.0        # (32,)
    w_cat = _pack_weights(W, s_o)

    nc = _get_nc()
    in_maps = []
    for core in range(N_CORES):
        vc = v[core * B_PER_CORE:(core + 1) * B_PER_CORE]
        xsh = vc.transpose(1, 2, 0, 3, 4).reshape(128, XFREE)
        in_maps.append({"x": np.ascontiguousarray(xsh), "w": w_cat})

    res = run_bass_kernel_spmd(
        nc, in_maps, core_ids=list(range(N_CORES)), trace=_trace
    )

    out = np.empty((B, C, L), dtype=np.float32)
    for core in range(N_CORES):
        o = res.results[core]["out"].reshape(C, JBLK, B_PER_CORE, NPH, NBLK)
        o = o.transpose(2, 0, 4, 1, 3).astype(np.float32)
        out[core * B_PER_CORE:(core + 1) * B_PER_CORE] = (
            o.reshape(B_PER_CORE, C, L) * s_o[None, :, None]
        )
    if _trace:
        return out, res
    return out
